# revision 24
# baseline (speedup 1.0000x reference)
"""DCNv2 (modulated deformable conv) Trainium2 Bass kernel, v2.

Shapes (hardcoded): x [4,128,128,64] f32, kernel [3,3,64,64], bias [64],
offset_kernel [3,3,64,27], offset_bias [27]. Output [4,128,128,64] f32.

Sharding: 8 cores = (batch 4) x (H halves 2). Each core computes 64 output
rows from a 72-row halo'd input slab (host-side zero-padded, pre-transposed
to channels-major bf16).

v2 architecture (per core):
  - host sends x pre-transposed/padded: xt [64c, 72*136] bf16 (+1-col
    shifted twin xt2 for 4B-aligned offset-conv reads).
  - offset conv om via accumulating PE matmuls -> om [27, px] bf16;
    om^T per output row via DMA xbar transposes -> omt [w, 64rho*27].
  - tent coefficients on DVE; written e-DUPLICATED: coef2[w,(k,rt,rho,e)]
    bf16 with e in {0,1} holding identical values, so the combiner's
    coefficient AP ends in a [stride 1, count 2] dim -> DVE 2x_1p mode.
  - per tap k: U_k = x @ W_k (PE, pointwise) -> PSUM -> SBUF (DVE/ACT
    copies); U^T via DMA xbar transposes (3 column shifts) -> ut
    [w, 36z*64co] bf16 per half-frame.
  - combiner per (k, rt): P = coef2-broadcast * ut-slice (DVE TT 2x);
    PE identity-matmul accumulates P chunks into a PSUM accumulator
    [w, 32rho*64co] f32 (bias pre-loaded via ones x biasrow matmul).
    A few rt per tap are offloaded to GPSIMD as classic per-rho
    scalar_tensor_tensor into an SBUF accumulator.
  - evac: ACT PSUM->SBUF, DVE adds the GPSIMD accumulator, DMA out NHWC.
  - processed in two 32-row half-frames to fit PSUM (4 banks acc + u psum).
"""

import numpy as np

B, H, W, C, CO = 4, 128, 128, 64, 64
KK = 9
PAD = 4
ROWS = 64 + 2 * PAD          # 72
WP = W + 2 * PAD             # 136
PX = ROWS * WP               # 9792
OUT_ROWS = 64
HH = 32                      # rho rows per half-frame
NZ = HH + 4                  # 36 u/ut rows per half
OPX = OUT_ROWS * WP          # 8704 offset-conv domain (padded rows 4..67)
OBASE = PAD * WP

# tuning knobs: (k, rt) terms handled by GPSIMD (TT mult+add)
POOL_TERMS = frozenset({(k, 4) for k in range(KK)}
                       | {(k, 2) for k in range(KK)}
                       | {(k, 6) for k in range(4)})
UCOPY_ENGINES = ("a",)  # engine per 1024-col u chunk

_CACHE = {}


def _build():
    import concourse.bass as bass  # noqa: F401
    import concourse.mybir as mybir
    from concourse.tile import TileContext
    from concourse.masks import make_identity

    OP = mybir.AluOpType
    AF = mybir.ActivationFunctionType
    f32 = mybir.dt.float32
    bf16 = mybir.dt.bfloat16

    nc = bass.Bass("TRN2")
    xt_d = nc.dram_tensor("xt", [C, PX], bf16, kind="ExternalInput")
    xt2_d = nc.dram_tensor("xt2", [C, PX], bf16, kind="ExternalInput")
    woff_d = nc.dram_tensor("woff", [C, KK * 32], f32, kind="ExternalInput")
    wmain_d = nc.dram_tensor("wmain", [C, KK * CO], f32, kind="ExternalInput")
    obias_d = nc.dram_tensor("obias", [32, 1], f32, kind="ExternalInput")
    brow_d = nc.dram_tensor("brow", [1, OUT_ROWS * CO], f32,
                            kind="ExternalInput")
    yout = nc.dram_tensor("yout", [OUT_ROWS * W, CO], f32,
                          kind="ExternalOutput")

    n_pool = len(POOL_TERMS)
    n_dve_terms = KK * KK - n_pool  # matmul-accumulated terms per half

    with TileContext(nc) as tc:
        with tc.tile_pool(name="persist", bufs=1) as pp:
            ident = pp.tile([128, 128], bf16)
            make_identity(nc, ident[:])
            woff_sb = pp.tile([C, KK * 32], bf16)
            nc.gpsimd.dma_start(out=woff_sb[:], in_=woff_d[:, :])
            wmain_sb = pp.tile([C, KK * CO], bf16)
            nc.gpsimd.dma_start(out=wmain_sb[:], in_=wmain_d[:, :])
            obias_sb = pp.tile([32, 1], f32)
            nc.sync.dma_start(out=obias_sb[:], in_=obias_d[:, :])
            brow_sb = pp.tile([1, OUT_ROWS * CO], bf16)
            nc.gpsimd.dma_start(out=brow_sb[:], in_=brow_d[:, :])
            x_sb = pp.tile([C, PX], bf16)
            x2_sb = pp.tile([C, PX], bf16)
            qs = (nc.sync, nc.gpsimd, nc.gpsimd)
            CHX = 24 * WP
            for ci in range(3):
                s = slice(ci * CHX, (ci + 1) * CHX)
                qs[ci % 3].dma_start(out=x_sb[:, s], in_=xt_d[:, s])
                qs[(ci + 1) % 3].dma_start(out=x2_sb[:, s], in_=xt2_d[:, s])
            ones_sb = pp.tile([1, 128], bf16)
            nc.gpsimd.memset(ones_sb[:], 1.0)

            om_sb = pp.tile([32, OPX], bf16)
            omt_sb = pp.tile([128, OUT_ROWS * 32], bf16)
            # coef2[w, ((k*9+rt)*64+rho)*2 + e]
            coef2 = pp.tile([128, KK * KK * OUT_ROWS * 2], bf16)
            pool_acc = pp.tile([128, OUT_ROWS * CO], bf16)
            if n_pool:
                nc.gpsimd.memset(pool_acc[:], 0.0)

            # ---- Stage B: offset conv (halved for pipelining) ----
            CH = 512
            with tc.tile_pool(name="omps", bufs=3, space="PSUM") as omp:
                def om_chunks(c_lo, c_hi):
                    for ci in range(c_lo, c_hi):
                        c0 = ci * CH
                        pom = omp.tile([32, CH], f32, name="pom", tag="pom")
                        for k in range(KK):
                            ky, kx = k // 3, k % 3
                            sh = (ky - 1) * WP + (kx - 1)
                            o = OBASE + c0 + sh
                            src = x_sb if (o % 2 == 0) else x2_sb
                            oo = o if (o % 2 == 0) else o - 1
                            nc.tensor.matmul(
                                pom[:], woff_sb[:, k * 32:k * 32 + 32],
                                src[:, oo:oo + CH],
                                start=(k == 0), stop=(k == KK - 1))
                        nc.scalar.activation(
                            om_sb[:, c0:c0 + CH], pom[:],
                            AF.Identity, bias=obias_sb[:, :], scale=1.0)

                def omt_half(h0):
                    for j, rho in enumerate(range(h0, h0 + HH)):
                        nc.sync.dma_start_transpose(
                            out=omt_sb[:, rho * 32:(rho + 1) * 32],
                            in_=om_sb[:, rho * WP + PAD: rho * WP + PAD + W])

                om_chunks(0, 9)
                omt_half(0)
                om_chunks(9, 17)
                omt_half(HH)

            # ---- Stage C: tents -> coef2 (issued per half) ----
            cw = ctx_cw = tc.tile_pool(name="coefw", bufs=1)
            cw = ctx_cw.__enter__()
            NF = HH * KK  # 288 per half

            def coef_half(h0):
                    om3 = omt_sb[:, h0 * 32:(h0 + HH) * 32].rearrange(
                        "p (r q) -> p r q", q=32)
                    dy_ap = om3[:, :, 0:9]
                    dx_ap = om3[:, :, 9:18]
                    mk_raw = om3[:, :, 18:27]
                    mk_sig = cw.tile([128, NF], f32, name="mks", tag="mks")
                    nc.scalar.activation(mk_sig[:], mk_raw, AF.Sigmoid)
                    tmp = cw.tile([128, NF], f32, name="tmp", tag="tmp")
                    tmpb = cw.tile([128, NF], f32, name="tmpb", tag="tmpb")
                    mty = [cw.tile([128, NF], f32, name=f"mty{r}",
                                   tag=f"mty{r}") for r in range(3)]
                    tx2 = [cw.tile([128, NF], f32, name=f"tx2{t}",
                                   tag=f"tx2{t}") for t in range(3)]
                    # tent(u-r) = relu(min(1-(u-r), 1+(u-r)))
                    for i, r in enumerate((-1, 0, 1)):
                        nc.vector.tensor_scalar(
                            out=tmp[:], in0=dy_ap, scalar1=-1.0,
                            scalar2=float(1 + r), op0=OP.mult, op1=OP.add)
                        nc.vector.tensor_scalar(
                            out=tmpb[:], in0=dy_ap, scalar1=float(1 - r),
                            scalar2=None, op0=OP.add)
                        nc.vector.tensor_tensor(
                            out=tmp[:], in0=tmp[:], in1=tmpb[:], op=OP.min)
                        nc.vector.scalar_tensor_tensor(
                            out=mty[i][:], in0=tmp[:], scalar=0.0,
                            in1=mk_sig[:], op0=OP.max, op1=OP.mult)
                    for i, t in enumerate((-1, 0, 1)):
                        nc.vector.tensor_scalar(
                            out=tmp[:], in0=dx_ap, scalar1=-1.0,
                            scalar2=float(1 + t), op0=OP.mult, op1=OP.add)
                        nc.vector.tensor_scalar(
                            out=tmpb[:], in0=dx_ap, scalar1=float(1 - t),
                            scalar2=None, op0=OP.add)
                        nc.vector.tensor_tensor(
                            out=tx2[i][:], in0=tmp[:], in1=tmpb[:], op=OP.min)
                    # coef2 writes: one 3D-AP STT per (k, rt) (walrus
                    # limits APs to partition + 2 free dims)
                    c5 = coef2[:].rearrange(
                        "p (k t r e) -> p k t r e", t=KK, r=OUT_ROWS, e=2)
                    tx3 = [t[:].rearrange("p (r q) -> p q r", q=KK)
                           for t in tx2]
                    mt3 = [m[:].rearrange("p (r q) -> p q r", q=KK)
                           for m in mty]
                    for k in range(KK):
                        for ri in range(3):
                            for ti in range(3):
                                rt = ri * 3 + ti
                                out_ap = c5[:, k, rt, h0:h0 + HH, :]
                                in0 = tx3[ti][:, k].unsqueeze(
                                    2).broadcast_to([128, HH, 2])
                                in1 = mt3[ri][:, k].unsqueeze(
                                    2).broadcast_to([128, HH, 2])
                                nc.vector.scalar_tensor_tensor(
                                    out=out_ap, in0=in0, scalar=0.0,
                                    in1=in1, op0=OP.max, op1=OP.mult)

            coef_half(0)

            # ---- per-half main pipeline ----
            c5 = coef2[:].rearrange(
                "p (j r e) -> p j r e", r=OUT_ROWS, e=2)  # j = k*9+rt
            ucols = NZ * WP       # 4896
            with tc.tile_pool(name="upool", bufs=2) as up, \
                 tc.tile_pool(name="utpool", bufs=2) as utp, \
                 tc.tile_pool(name="ppool", bufs=4) as ppl, \
                 tc.tile_pool(name="plpool", bufs=2) as plp, \
                 tc.tile_pool(name="fin", bufs=2) as fp, \
                 tc.tile_pool(name="ups", bufs=2, space="PSUM") as upp, \
                 tc.tile_pool(name="accps", bufs=1, space="PSUM") as acp:
              for h0 in (0, HH):
                z0 = h0 + 2           # first padded row of the u/ut frame
                ubase = z0 * WP
                if True:
                    acc = acp.tile([128, HH * CO], f32, name="acc",
                                   tag="acc")
                    for cc in range(4):
                        nc.tensor.matmul(
                            acc[:, cc * 512:(cc + 1) * 512], ones_sb[:],
                            brow_sb[:, h0 * CO + cc * 512:
                                    h0 * CO + (cc + 1) * 512],
                            start=True, stop=False)

                    def make_u(k):
                        u_k = up.tile([C, ucols], bf16, name="u", tag="u")
                        for pi, p0 in enumerate(range(0, ucols, 1024)):
                            cw_ = min(1024, ucols - p0)
                            pu = upp.tile([C, 1024], f32, name="pu",
                                          tag="pu")
                            for q0 in range(0, cw_, 512):
                                qw = min(512, cw_ - q0)
                                nc.tensor.matmul(
                                    pu[:, q0:q0 + qw],
                                    wmain_sb[:, k * CO:(k + 1) * CO],
                                    x_sb[:, ubase + p0 + q0:
                                         ubase + p0 + q0 + qw],
                                    start=True, stop=True)
                            if UCOPY_ENGINES[pi % len(UCOPY_ENGINES)] == "v":
                                nc.vector.tensor_copy(
                                    u_k[:, p0:p0 + cw_], pu[:, :cw_])
                            else:
                                nc.scalar.activation(
                                    u_k[:, p0:p0 + cw_], pu[:, :cw_],
                                    AF.Copy)
                        return u_k

                    def make_ut(k, u_k):
                        kx = k % 3
                        uts = []
                        for ti in range(3):
                            dcol = kx + ti - 2
                            ut = utp.tile([128, NZ * CO], bf16,
                                          name=f"ut{ti}", tag=f"ut{ti}")
                            uts.append(ut)
                            for zi in range(NZ):
                                nc.sync.dma_start_transpose(
                                    out=ut[:, zi * CO:(zi + 1) * CO],
                                    in_=u_k[:, zi * WP + PAD + dcol:
                                            zi * WP + PAD + dcol + W])
                        return uts

                    dve_idx = 0
                    u_k = make_u(0)
                    uts = make_ut(0, u_k)
                    for k in range(KK):
                        ky = k // 3
                        if h0 == 0 and k == 2:
                            coef_half(HH)
                        if k + 1 < KK:
                            u_next = make_u(k + 1)
                        for ri in range(3):
                            drow = ky - 1 + ri - 1
                            for ti in range(3):
                                rt = ri * 3 + ti
                                j = k * KK + rt
                                ut = uts[ti]
                                if (k, rt) in POOL_TERMS:
                                    Ppl = plp.tile([128, 2048], bf16,
                                                   name="Ppl", tag="Ppl")
                                    zlo = 2 + drow
                                    in1 = ut[:, zlo * CO:(zlo + HH) * CO]
                                    in0 = c5[:, j, h0:h0 + HH, :]\
                                        .unsqueeze(2).broadcast_to(
                                            [128, HH, 32, 2])
                                    outP = Ppl[:].rearrange(
                                        "p (r c e) -> p r c e", c=32, e=2)
                                    in1v = in1.rearrange(
                                        "p (r c e) -> p r c e", c=32, e=2)
                                    nc.gpsimd.tensor_tensor(
                                        out=outP, in0=in0, in1=in1v,
                                        op=OP.mult)
                                    seg = slice(h0 * CO, h0 * CO + HH * CO)
                                    nc.gpsimd.tensor_tensor(
                                        out=pool_acc[:, seg], in0=Ppl[:],
                                        in1=pool_acc[:, seg], op=OP.add)
                                    continue
                                dve_idx += 1
                                last = dve_idx == n_dve_terms
                                P = ppl.tile([128, 2048], bf16,
                                             name="P", tag="P")
                                zlo = 2 + drow
                                in1 = ut[:, zlo * CO:(zlo + HH) * CO]
                                in0 = c5[:, j, h0:h0 + HH, :]\
                                    .unsqueeze(2).broadcast_to(
                                        [128, HH, 32, 2])
                                outP = P[:].rearrange(
                                    "p (r c e) -> p r c e", c=32, e=2)
                                in1v = in1.rearrange(
                                    "p (r c e) -> p r c e", c=32, e=2)
                                nc.vector.tensor_tensor(
                                    out=outP, in0=in0, in1=in1v,
                                    op=OP.mult)
                                for q in range(4):
                                    nc.tensor.matmul(
                                        acc[:, q * 512:(q + 1) * 512],
                                        ident[:],
                                        P[:, q * 512:(q + 1) * 512],
                                        start=False, stop=False)
                        if k + 1 < KK:
                            uts = make_ut(k + 1, u_next)
                            u_k = u_next

                    # ---- merge pool acc into psum (PE) + evac ----
                    yo3 = yout.rearrange(
                        "(r w) c -> r w c", w=W).transpose([1, 0, 2])
                    for cc in range(4):
                        nc.tensor.matmul(
                            acc[:, cc * 512:(cc + 1) * 512], ident[:],
                            pool_acc[:, h0 * CO + cc * 512:
                                     h0 * CO + (cc + 1) * 512],
                            start=False, stop=True)
                        st = fp.tile([128, 512], f32, name="st",
                                     tag="st")
                        nc.scalar.activation(
                            st[:], acc[:, cc * 512:(cc + 1) * 512],
                            AF.Copy)
                        nc.sync.dma_start(
                            out=yo3[:, h0 + cc * 8: h0 + (cc + 1) * 8, :],
                            in_=st[:].rearrange("w (r c) -> w r c", r=8))
            ctx_cw.__exit__(None, None, None)

    return nc


def _split_multi_waits(nc, mybir, bass_rust):
    """This walrus accepts only one sync wait per instruction; move extras
    onto same-engine NoOps placed immediately before."""
    ctr = 0
    for fn in nc.m.functions:
        for bb in fn.blocks:
            new_insts = []
            for inst in bb.instructions:
                si = inst.sync_info
                if si is not None and len(si.on_wait) > 1:
                    waits = list(si.on_wait)
                    for w in waits[:-1]:
                        ctr += 1
                        nop = mybir.InstNoOp(name=f"I-waitsplit-{ctr}")
                        nop.engine = inst.engine
                        nop.sync_info = bass_rust.SyncInfo(
                            on_wait=[w], on_update=[])
                        new_insts.append(nop)
                    inst.sync_info = bass_rust.SyncInfo(
                        on_wait=[waits[-1]], on_update=list(si.on_update))
                new_insts.append(inst)
            bb.instructions = new_insts


def _get_nc(split=True):
    key = ("nc", split)
    if key not in _CACHE:
        import concourse.mybir as mybir
        import bass_rust
        nc = _build()
        if split:
            _split_multi_waits(nc, mybir, bass_rust)
        _CACHE[key] = nc
    return _CACHE[key]


def _tobf16(a):
    import jax.numpy as jnp
    return np.asarray(jnp.asarray(np.asarray(a), jnp.bfloat16))


def make_in_maps(x, kernel, bias, offset_kernel, offset_bias):
    x = np.ascontiguousarray(np.asarray(x), np.float32)
    perm = list(range(0, 18, 2)) + list(range(1, 18, 2)) + list(range(18, 27))
    w0 = np.asarray(offset_kernel).reshape(KK, C, 27)[:, :, perm]
    w0 = np.concatenate([w0, np.zeros((KK, C, 5), w0.dtype)], axis=2)
    woff = np.ascontiguousarray(
        w0.transpose(1, 0, 2).reshape(C, KK * 32), np.float32)
    wmain = np.ascontiguousarray(
        np.asarray(kernel).reshape(KK, C, CO)
        .transpose(1, 0, 2).reshape(C, KK * CO), np.float32)
    obias = np.ascontiguousarray(
        np.concatenate([np.asarray(offset_bias)[perm],
                        np.zeros(5, np.float32)]).reshape(32, 1),
        np.float32)
    brow = np.ascontiguousarray(
        np.tile(np.asarray(bias, np.float32).reshape(1, CO),
                (OUT_ROWS, 1)).reshape(1, OUT_ROWS * CO))
    in_maps = []
    xbf = _tobf16(x)  # [B, H, W, C] bf16
    for core in range(8):
        b, half = core // 2, core % 2
        h0 = half * 64
        slabT = np.zeros((C, ROWS, WP), xbf.dtype)
        lo, hi = h0 - PAD, h0 + 64 + PAD
        slo, shi = max(lo, 0), min(hi, H)
        slabT[:, slo - lo: shi - lo, PAD:PAD + W] = \
            xbf[b, slo:shi].transpose(2, 0, 1)
        xt = np.ascontiguousarray(slabT.reshape(C, PX))
        xt2 = np.zeros_like(xt)
        xt2[:, :-1] = xt[:, 1:]
        in_maps.append({
            "xt": xt, "xt2": xt2,
            "woff": woff, "wmain": wmain,
            "obias": obias, "brow": brow,
        })
    return in_maps


def run(x, kernel, bias, offset_kernel, offset_bias, **kwargs):
    from concourse.bass_utils import run_bass_kernel_spmd
    nc = _get_nc()
    in_maps = make_in_maps(x, kernel, bias, offset_kernel, offset_bias)
    res = run_bass_kernel_spmd(nc, in_maps, core_ids=list(range(8)), **kwargs)
    out = np.empty((B, H, W, CO), np.float32)
    for core in range(8):
        b, half = core // 2, core % 2
        out[b, half * 64:half * 64 + 64] = (
            res.results[core]["yout"].reshape(64, W, CO))
    return out, res


def kernel(**inputs):
    out, _ = run(**inputs)
    return out
